# revision 2
# baseline (speedup 1.0000x reference)
"""AdaptiveCenterLoss on 8 TRN2 NeuronCores — fp8 flipped-layout version.

loss = mean_i ||features[i] - centers[labels[i]]||^2, B=131072 D=256 C=1000.

Strategy (memory-bound; tolerance 2e-2 >> fp8 e4m3 error ~1e-3):
  loss*B = sum(f^2) - 2*sum_i f_i.c_{l_i} + sum_i ||c_{l_i}||^2
  - features ship as fp8 e4m3 (4.2MB/core instead of 8.4MB bf16).
  - layout is FLIPPED vs the bf16 kernel: rows sit on PARTITIONS.  Each
    tile is [128 rows x 256 d]; host sorts rows by label and packs them
    so each tile is made of single-label blocks of s rows (s in
    {16,8,4,2,1} by binary decomposition of class counts; pad rows equal
    the class-0 center so they cancel exactly).
  - TensorE does ALL the heavy math in one pass: per 128-col chunk X,
    matmul(X, X) accumulates a Gram whose diag sums to sum(f^2), and a
    tiny matmul(X, ones_blocks[128,k]) emits per-block column sums
    (k = 128/s).  Both land in the SAME PSUM bank ("combo" layout:
    cols 0:128 gram, 128:512 colsums) so back-to-back matmuls never
    switch PSUM banks — bank switches were measured to cost ~25ns each
    on the MM pipe.  Tiles are grouped into 7 stretches, one bank each.
  - When a stretch completes, DVE dots its colsums against the per-block
    centers (fp8, host-materialized in PSUM column order) and extracts
    its partial Gram diagonal via an identity mask — work spreads
    through the run instead of piling at the drain.
  - host computes sum ||c||^2 exactly in fp64 and combines.
"""

import numpy as np
import ml_dtypes

import concourse.bacc as bacc
import concourse.bass as bass
import concourse.mybir as mybir
import concourse.tile as tile
from concourse.bass_utils import run_bass_kernel_spmd

B, D, C = 131072, 256, 1000
N_CORES = 8
P = 128
SIZES = (1, 2, 4, 8, 16)  # region order on device (expensive colsums first)
FP8 = ml_dtypes.float8_e4m3
CSCAP = 512  # colsum columns per PSUM bank

_nc_cache = {}


def _slice_plan(T):
    """DMA slices: small first (compute starts early), big middle,
    small last (short drain tail)."""
    sizes = []
    rem = T
    for want in (2, 4, 6, 8, 12, 16, 20):
        if rem <= 0:
            break
        take = min(want, rem)
        sizes.append(take)
        rem -= take
    mid = []
    while rem > 18:
        take = min(24, rem - 12)
        mid.append(take)
        rem -= take
    tail = []
    while rem > 0:
        take = min(8, rem)
        tail.append(take)
        rem -= take
    return sizes + mid + sorted(tail, reverse=True)


def _banks(tile_size):
    """Greedy stretch assignment: per tile, its 2k colsum cols must fit
    in the current bank's colsum capacity; else open a new bank.
    Returns per-tile (bank, col offset within bank's colsum area)."""
    assign = []
    bank, off = 0, 0
    for s in tile_size:
        w = 2 * (P // s)
        if off + w > CSCAP:
            bank += 1
            off = 0
        assign.append((bank, off))
        off += w
    return assign, bank + 1


def _build(cfg):
    if cfg in _nc_cache:
        return _nc_cache[cfg]
    t_by_size = dict(zip(SIZES, cfg))
    T = sum(t_by_size.values())
    tile_size = []
    for s in SIZES:
        tile_size += [s] * t_by_size[s]
    assign, NBANK = _banks(tile_size)
    assert NBANK <= 7, f"{NBANK} colsum banks + gram exceed PSUM"
    OW = sum(P // s for s in SIZES if t_by_size[s])  # ones pack width
    slices = _slice_plan(T)
    # colsum column counts per bank (for cents slicing)
    bank_w = [0] * NBANK
    for (b, off), s in zip(assign, tile_size):
        bank_w[b] = max(bank_w[b], off + 2 * (P // s))
    bank_c0 = np.concatenate(([0], np.cumsum(bank_w)[:-1])).tolist()
    CC = int(sum(bank_w))

    nc = bacc.Bacc()
    feats = nc.declare_dram_parameter(
        "features", [P, T * D], mybir.dt.float8e4, isOutput=False
    )
    cents = nc.declare_dram_parameter(
        "cents", [P, CC], mybir.dt.float8e4, isOutput=False
    )
    onesp = nc.declare_dram_parameter(
        "onesp", [P, OW], mybir.dt.float8e4, isOutput=False
    )
    ident = nc.declare_dram_parameter("ident", [P, P], mybir.dt.bfloat16, isOutput=False)
    out = nc.declare_dram_parameter("out", [P, 16], mybir.dt.float32, isOutput=True)

    # ones views offsets per size
    ones_off = {}
    o = 0
    for s in SIZES:
        if t_by_size[s]:
            ones_off[s] = o
            o += P // s

    with tile.TileContext(nc) as tc:
        with (
            tc.tile_pool(name="c", bufs=1) as c_pool,
            tc.tile_pool(name="x", bufs=len(slices)) as x_pool,
            tc.tile_pool(name="scr", bufs=2) as scr_pool,
            tc.tile_pool(name="ps", bufs=1, space=bass.MemorySpace.PSUM) as ps_pool,
        ):
            ones_t = c_pool.tile([P, OW], mybir.dt.float8e4)
            nc.scalar.dma_start(out=ones_t[:], in_=onesp[:])
            cents_t = c_pool.tile([P, CC], mybir.dt.float8e4)
            nc.scalar.dma_start(out=cents_t[:], in_=cents[:])
            idt = c_pool.tile([P, P], mybir.dt.bfloat16)
            nc.scalar.dma_start(out=idt[:], in_=ident[:])
            acc = c_pool.tile([P, 16], mybir.dt.float32)

            ps_gram = ps_pool.tile([P, P], mybir.dt.float32, tag="g")
            combo = [
                ps_pool.tile([P, 512], mybir.dt.float32, tag=f"cb{i}", name=f"cb{i}")
                for i in range(NBANK)
            ]

            # feature slices
            xs = []
            t0 = 0
            for i, nt in enumerate(slices):
                xt = x_pool.tile([P, nt * D], mybir.dt.float8e4, tag="x", name=f"x{i}")
                nc.sync.dma_start(out=xt[:], in_=feats[:, t0 * D : (t0 + nt) * D])
                xs.append((t0, nt, xt))
                t0 += nt

            # first/last tile of each bank stretch
            first_of = {}
            last_of = {}
            for tl, (b, _) in enumerate(assign):
                if b not in first_of:
                    first_of[b] = tl
                last_of[b] = tl

            def finish_bank(b):
                w = bank_w[b]
                scr = scr_pool.tile([P, 512], mybir.dt.float32, tag="scr")
                nc.vector.scalar_tensor_tensor(
                    out=scr[:, :w],
                    in0=combo[b][:, :w],
                    scalar=0.0,
                    in1=cents_t[:, bank_c0[b] : bank_c0[b] + w],
                    op0=mybir.AluOpType.bypass,
                    op1=mybir.AluOpType.mult,
                    accum_out=acc[:, 2 * b + 1 : 2 * b + 2],
                )

            nchunks = 2 * T
            ci = 0
            for t0_, nt, xt in xs:
                for tl in range(t0_, t0_ + nt):
                    s = tile_size[tl]
                    k = P // s
                    base = (tl - t0_) * D
                    b, off = assign[tl]
                    for h in range(2):
                        chunk = xt[:, base + h * P : base + (h + 1) * P]
                        nc.tensor.matmul(
                            ps_gram[:],
                            chunk,
                            chunk,
                            start=(ci == 0),
                            stop=(ci == nchunks - 1),
                        )
                        ci += 1
                        nc.tensor.matmul(
                            combo[b][:, off + h * k : off + (h + 1) * k],
                            chunk,
                            ones_t[:, ones_off[s] : ones_off[s] + k],
                            start=True,
                            stop=True,
                        )
                    if tl == last_of[b]:
                        finish_bank(b)
            # gram diagonal
            scr2 = scr_pool.tile([P, P], mybir.dt.float32, tag="scr2")
            nc.vector.scalar_tensor_tensor(
                out=scr2[:],
                in0=ps_gram[:],
                scalar=0.0,
                in1=idt[:],
                op0=mybir.AluOpType.bypass,
                op1=mybir.AluOpType.mult,
                accum_out=acc[:, 0:1],
            )
            nc.sync.dma_start(out=out[:], in_=acc[:])
    nc.finalize()
    _nc_cache[cfg] = (nc, NBANK)
    return nc, NBANK


def _prepare(features, centers, labels):
    features = np.asarray(features)
    centers_f = np.ascontiguousarray(np.asarray(centers), dtype=np.float32)
    c_q = centers_f.astype(FP8)  # quantized centers (used consistently)
    c_qf = c_q.astype(np.float64)
    labels = np.asarray(labels).astype(np.int64)

    counts = np.bincount(labels, minlength=C).astype(np.int64)
    nb = {16: counts // 16}
    rem = counts % 16
    nb[8] = (rem // 8) % 2
    nb[4] = (rem // 4) % 2
    nb[2] = (rem // 2) % 2
    nb[1] = rem % 2

    # per-core tile counts, padded block lists (pad class = 0)
    t_by_size = {}
    bl = {}
    for s in SIZES:
        K = P // s
        N_s = int(nb[s].sum())
        t_s = -(-N_s // (N_CORES * K)) if N_s else 0
        t_by_size[s] = t_s
        cap = N_CORES * t_s * K
        b = np.repeat(np.arange(C, dtype=np.int64), nb[s])
        if cap > len(b):
            b = np.concatenate([b, np.zeros(cap - len(b), dtype=np.int64)])
        bl[s] = b
    T = sum(t_by_size.values())
    cfg = tuple(t_by_size[s] for s in SIZES)
    tile_size = []
    for s in SIZES:
        tile_size += [s] * t_by_size[s]
    assign, NBANK = _banks(tile_size)

    # region tile offsets (per core), region order = SIZES
    region_off = {}
    o = 0
    for s in SIZES:
        region_off[s] = o
        o += t_by_size[s]

    # ---- row destinations ----
    order_idx = np.argsort(labels, kind="stable")
    labels_sorted = labels[order_idx]
    class_row_start = np.concatenate(([0], np.cumsum(counts)[:-1]))
    rank = np.arange(B, dtype=np.int64) - class_row_start[labels_sorted]

    dst = np.empty(B, dtype=np.int64)
    lo = np.zeros(C, dtype=np.int64)
    for s in SIZES:
        ns = nb[s]
        hi = lo + s * ns
        m = (rank >= lo[labels_sorted]) & (rank < hi[labels_sorted])
        if m.any():
            K = P // s
            j = labels_sorted[m]
            r = rank[m] - lo[j]
            start_s = np.concatenate(([0], np.cumsum(ns)[:-1]))
            bidx = start_s[j] + r // s  # global block index in bl[s]
            per_core = t_by_size[s] * K
            core = bidx // per_core
            loc = bidx % per_core
            tile_loc = loc // K
            bslot = loc % K
            part = bslot * s + r % s
            tile_glob = region_off[s] + tile_loc
            dst[m] = (core * P + part) * T + tile_glob
        lo = hi

    f_q = features.astype(np.float32).astype(FP8)
    fpad = np.empty((N_CORES * P * T, D), dtype=FP8)
    fpad[:] = c_q[0]  # pad rows cancel exactly
    fpad[dst] = f_q[order_idx]

    # ---- centsT (per-core, PSUM combo-bank column order) ----
    # within a bank, columns follow tile order (h0 k cols then h1 k cols),
    # padded to bank_w at the end of the bank; pad cols get 0 centers
    # (their PSUM cells are never written... but also never dotted since
    # bank_w only covers written columns).
    lab_by_size = {}
    for s in SIZES:
        if t_by_size[s]:
            lab_by_size[s] = bl[s].reshape(N_CORES, t_by_size[s], P // s)
    tinfo = []
    for s in SIZES:
        for i in range(t_by_size[s]):
            tinfo.append((s, i))

    cents_maps = []
    for core in range(N_CORES):
        parts = []
        for tl in range(T):
            s, iloc = tinfo[tl]
            K = P // s
            lab = lab_by_size[s][core, iloc]  # [K]
            M = c_q[lab]  # [K, 256]
            M2 = M.reshape(K, 2, P).transpose(2, 1, 0)  # [128, 2, K]
            parts.append(M2.reshape(P, 2 * K))
        cents_maps.append(np.ascontiguousarray(np.concatenate(parts, axis=1)))

    # ---- ones pack: per size [128, K] block-indicator ----
    ow = []
    for s in SIZES:
        if t_by_size[s] == 0:
            continue
        K = P // s
        blk = np.zeros((P, K), dtype=FP8)
        for jj in range(K):
            blk[jj * s : (jj + 1) * s, jj] = 1.0
        ow.append(blk)
    onesp = np.ascontiguousarray(np.concatenate(ow, axis=1))

    ident = np.eye(P, dtype=ml_dtypes.bfloat16)

    maps = []
    fview = fpad.reshape(N_CORES, P, T * D)
    for core in range(N_CORES):
        maps.append(
            {
                "features": np.ascontiguousarray(fview[core]),
                "cents": cents_maps[core],
                "onesp": onesp,
                "ident": ident,
            }
        )

    # ---- host-exact center term over all device rows (incl pads) ----
    cn2 = (c_qf * c_qf).sum(axis=1)  # fp64 ||c_q||^2 per class
    c2_total = 0.0
    for s in SIZES:
        if len(bl[s]):
            c2_total += s * float(cn2[bl[s]].sum())

    return maps, cfg, c2_total


def run(features, centers, labels, trace=False):
    maps, cfg, c2_total = _prepare(features, centers, labels)
    nc, NBANK = _build(cfg)
    res = run_bass_kernel_spmd(nc, maps, core_ids=list(range(N_CORES)), trace=trace)
    gram = 0.0
    cross = 0.0
    for r in res.results:
        o = np.asarray(r["out"]).astype(np.float64)
        gram += float(o[:, 0].sum())
        cross += float(o[:, 1 : 2 * NBANK : 2].sum())
    total = gram - 2.0 * cross + c2_total
    return np.float32(total / B), res


def kernel(features, centers, labels):
    last_err = None
    for _ in range(3):
        try:
            loss, _ = run(features, centers, labels)
            return loss
        except Exception as e:  # noqa: BLE001
            last_err = e
    raise last_err


# revision 3
# speedup vs baseline: 1.0658x; 1.0658x over previous
"""AdaptiveCenterLoss on 8 TRN2 NeuronCores — fp8 flipped-layout version.

loss = mean_i ||features[i] - centers[labels[i]]||^2, B=131072 D=256 C=1000.

Strategy (memory-bound; tolerance 2e-2 >> fp8 e4m3 error ~1e-3):
  loss*B = sum(f^2) - 2*sum_i f_i.c_{l_i} + sum_i ||c_{l_i}||^2
  - features ship as fp8 e4m3 (4.2MB/core instead of 8.4MB bf16).
  - layout is FLIPPED vs the bf16 kernel: rows sit on PARTITIONS.  Each
    tile is [128 rows x 256 d]; host sorts rows by label and packs them
    so each tile is made of single-label blocks of s rows (s in
    {16,8,4,2,1} by binary decomposition of class counts; pad rows equal
    the class-0 center so they cancel exactly).
  - TensorE does ALL the heavy math in one pass: per 128-col chunk X,
    matmul(X, X) accumulates a Gram whose diag sums to sum(f^2), and a
    tiny matmul(X, ones_blocks[128,k]) emits per-block column sums
    (k = 128/s).  Both land in the SAME PSUM bank ("combo" layout:
    cols 0:128 gram, 128:512 colsums) so back-to-back matmuls never
    switch PSUM banks — bank switches were measured to cost ~25ns each
    on the MM pipe.  Tiles are grouped into 7 stretches, one bank each.
  - When a stretch completes, DVE dots its colsums against the per-block
    centers (fp8, host-materialized in PSUM column order) and extracts
    its partial Gram diagonal via an identity mask — work spreads
    through the run instead of piling at the drain.
  - host computes sum ||c||^2 exactly in fp64 and combines.
"""

import numpy as np
import ml_dtypes

import concourse.bacc as bacc
import concourse.bass as bass
import concourse.mybir as mybir
import concourse.tile as tile
from concourse.bass_utils import run_bass_kernel_spmd

B, D, C = 131072, 256, 1000
N_CORES = 8
P = 128
SIZES = (1, 2, 4, 8, 16)  # region order on device (expensive colsums first)
FP8 = ml_dtypes.float8_e4m3
CSCAP = 512  # colsum columns per PSUM bank

_nc_cache = {}


def _slice_plan(T):
    """DMA slices: small first (compute starts early), big middle,
    small last (short drain tail)."""
    sizes = []
    rem = T
    for want in (3, 5, 8, 12, 16, 20):
        if rem <= 0:
            break
        take = min(want, rem)
        sizes.append(take)
        rem -= take
    mid = []
    while rem > 18:
        take = min(24, rem - 12)
        mid.append(take)
        rem -= take
    tail = []
    while rem > 0:
        take = min(8, rem)
        tail.append(take)
        rem -= take
    return sizes + mid + sorted(tail, reverse=True)


def _banks(tile_size):
    """Greedy stretch assignment: per tile, its 2k colsum cols must fit
    in the current bank's colsum capacity; else open a new bank.
    Returns per-tile (bank, col offset within bank's colsum area)."""
    assign = []
    bank, off = 0, 0
    for s in tile_size:
        w = 2 * (P // s)
        if off + w > CSCAP:
            bank += 1
            off = 0
        assign.append((bank, off))
        off += w
    return assign, bank + 1


def _build(cfg):
    if cfg in _nc_cache:
        return _nc_cache[cfg]
    t_by_size = dict(zip(SIZES, cfg))
    T = sum(t_by_size.values())
    tile_size = []
    for s in SIZES:
        tile_size += [s] * t_by_size[s]
    assign, NBANK = _banks(tile_size)
    assert NBANK <= 7, f"{NBANK} colsum banks + gram exceed PSUM"
    OW = sum(P // s for s in SIZES if t_by_size[s])  # ones pack width
    slices = _slice_plan(T)
    # colsum column counts per bank (for cents slicing)
    bank_w = [0] * NBANK
    for (b, off), s in zip(assign, tile_size):
        bank_w[b] = max(bank_w[b], off + 2 * (P // s))
    bank_c0 = np.concatenate(([0], np.cumsum(bank_w)[:-1])).tolist()
    CC = int(sum(bank_w))

    nc = bacc.Bacc()
    feats = nc.declare_dram_parameter(
        "features", [P, T * D], mybir.dt.float8e4, isOutput=False
    )
    cents = nc.declare_dram_parameter(
        "cents", [P, CC], mybir.dt.float8e4, isOutput=False
    )
    onesp = nc.declare_dram_parameter(
        "onesp", [P, OW], mybir.dt.float8e4, isOutput=False
    )
    ident = nc.declare_dram_parameter("ident", [P, P], mybir.dt.bfloat16, isOutput=False)
    out = nc.declare_dram_parameter("out", [P, 16], mybir.dt.float32, isOutput=True)

    # ones views offsets per size
    ones_off = {}
    o = 0
    for s in SIZES:
        if t_by_size[s]:
            ones_off[s] = o
            o += P // s

    with tile.TileContext(nc) as tc:
        with (
            tc.tile_pool(name="c", bufs=1) as c_pool,
            tc.tile_pool(name="x", bufs=len(slices)) as x_pool,
            tc.tile_pool(name="scr", bufs=2) as scr_pool,
            tc.tile_pool(name="ps", bufs=1, space=bass.MemorySpace.PSUM) as ps_pool,
        ):
            ones_t = c_pool.tile([P, OW], mybir.dt.float8e4)
            nc.scalar.dma_start(out=ones_t[:], in_=onesp[:])
            idt = c_pool.tile([P, P], mybir.dt.bfloat16)
            nc.scalar.dma_start(out=idt[:], in_=ident[:])
            cents_t = c_pool.tile([P, CC], mybir.dt.float8e4)
            nc.scalar.dma_start(out=cents_t[:], in_=cents[:])
            acc = c_pool.tile([P, 16], mybir.dt.float32)

            ps_gram = ps_pool.tile([P, P], mybir.dt.float32, tag="g")
            combo = [
                ps_pool.tile([P, 512], mybir.dt.float32, tag=f"cb{i}", name=f"cb{i}")
                for i in range(NBANK)
            ]

            # feature slices
            xs = []
            t0 = 0
            for i, nt in enumerate(slices):
                xt = x_pool.tile([P, nt * D], mybir.dt.float8e4, tag="x", name=f"x{i}")
                nc.sync.dma_start(out=xt[:], in_=feats[:, t0 * D : (t0 + nt) * D])
                xs.append((t0, nt, xt))
                t0 += nt

            # first/last tile of each bank stretch
            first_of = {}
            last_of = {}
            for tl, (b, _) in enumerate(assign):
                if b not in first_of:
                    first_of[b] = tl
                last_of[b] = tl

            def finish_bank(b):
                w = bank_w[b]
                scr = scr_pool.tile([P, 512], mybir.dt.float32, tag="scr")
                nc.vector.scalar_tensor_tensor(
                    out=scr[:, :w],
                    in0=combo[b][:, :w],
                    scalar=0.0,
                    in1=cents_t[:, bank_c0[b] : bank_c0[b] + w],
                    op0=mybir.AluOpType.bypass,
                    op1=mybir.AluOpType.mult,
                    accum_out=acc[:, 2 * b + 1 : 2 * b + 2],
                )

            nchunks = 2 * T
            ci = 0
            for t0_, nt, xt in xs:
                for tl in range(t0_, t0_ + nt):
                    s = tile_size[tl]
                    k = P // s
                    base = (tl - t0_) * D
                    b, off = assign[tl]
                    for h in range(2):
                        chunk = xt[:, base + h * P : base + (h + 1) * P]
                        nc.tensor.matmul(
                            ps_gram[:],
                            chunk,
                            chunk,
                            start=(ci == 0),
                            stop=(ci == nchunks - 1),
                        )
                        ci += 1
                        nc.tensor.matmul(
                            combo[b][:, off + h * k : off + (h + 1) * k],
                            chunk,
                            ones_t[:, ones_off[s] : ones_off[s] + k],
                            start=True,
                            stop=True,
                        )
                    if tl == last_of[b]:
                        finish_bank(b)
            # gram diagonal
            scr2 = scr_pool.tile([P, P], mybir.dt.float32, tag="scr2")
            nc.vector.scalar_tensor_tensor(
                out=scr2[:],
                in0=ps_gram[:],
                scalar=0.0,
                in1=idt[:],
                op0=mybir.AluOpType.bypass,
                op1=mybir.AluOpType.mult,
                accum_out=acc[:, 0:1],
            )
            nc.sync.dma_start(out=out[:], in_=acc[:])
    nc.finalize()
    _nc_cache[cfg] = (nc, NBANK)
    return nc, NBANK


def _prepare(features, centers, labels):
    features = np.asarray(features)
    centers_f = np.ascontiguousarray(np.asarray(centers), dtype=np.float32)
    c_q = centers_f.astype(FP8)  # quantized centers (used consistently)
    c_qf = c_q.astype(np.float64)
    labels = np.asarray(labels).astype(np.int64)

    counts = np.bincount(labels, minlength=C).astype(np.int64)
    nb = {16: counts // 16}
    rem = counts % 16
    nb[8] = (rem // 8) % 2
    nb[4] = (rem // 4) % 2
    nb[2] = (rem // 2) % 2
    nb[1] = rem % 2

    # per-core tile counts, padded block lists (pad class = 0)
    t_by_size = {}
    bl = {}
    for s in SIZES:
        K = P // s
        N_s = int(nb[s].sum())
        t_s = -(-N_s // (N_CORES * K)) if N_s else 0
        t_by_size[s] = t_s
        cap = N_CORES * t_s * K
        b = np.repeat(np.arange(C, dtype=np.int64), nb[s])
        if cap > len(b):
            b = np.concatenate([b, np.zeros(cap - len(b), dtype=np.int64)])
        bl[s] = b
    T = sum(t_by_size.values())
    cfg = tuple(t_by_size[s] for s in SIZES)
    tile_size = []
    for s in SIZES:
        tile_size += [s] * t_by_size[s]
    assign, NBANK = _banks(tile_size)

    # region tile offsets (per core), region order = SIZES
    region_off = {}
    o = 0
    for s in SIZES:
        region_off[s] = o
        o += t_by_size[s]

    # ---- row destinations ----
    order_idx = np.argsort(labels, kind="stable")
    labels_sorted = labels[order_idx]
    class_row_start = np.concatenate(([0], np.cumsum(counts)[:-1]))
    rank = np.arange(B, dtype=np.int64) - class_row_start[labels_sorted]

    dst = np.empty(B, dtype=np.int64)
    lo = np.zeros(C, dtype=np.int64)
    for s in SIZES:
        ns = nb[s]
        hi = lo + s * ns
        m = (rank >= lo[labels_sorted]) & (rank < hi[labels_sorted])
        if m.any():
            K = P // s
            j = labels_sorted[m]
            r = rank[m] - lo[j]
            start_s = np.concatenate(([0], np.cumsum(ns)[:-1]))
            bidx = start_s[j] + r // s  # global block index in bl[s]
            per_core = t_by_size[s] * K
            core = bidx // per_core
            loc = bidx % per_core
            tile_loc = loc // K
            bslot = loc % K
            part = bslot * s + r % s
            tile_glob = region_off[s] + tile_loc
            dst[m] = (core * P + part) * T + tile_glob
        lo = hi

    f_q = features.astype(np.float32).astype(FP8)
    fpad = np.empty((N_CORES * P * T, D), dtype=FP8)
    fpad[:] = c_q[0]  # pad rows cancel exactly
    fpad[dst] = f_q[order_idx]

    # ---- centsT (per-core, PSUM combo-bank column order) ----
    # within a bank, columns follow tile order (h0 k cols then h1 k cols),
    # padded to bank_w at the end of the bank; pad cols get 0 centers
    # (their PSUM cells are never written... but also never dotted since
    # bank_w only covers written columns).
    lab_by_size = {}
    for s in SIZES:
        if t_by_size[s]:
            lab_by_size[s] = bl[s].reshape(N_CORES, t_by_size[s], P // s)
    tinfo = []
    for s in SIZES:
        for i in range(t_by_size[s]):
            tinfo.append((s, i))

    cents_maps = []
    for core in range(N_CORES):
        parts = []
        for tl in range(T):
            s, iloc = tinfo[tl]
            K = P // s
            lab = lab_by_size[s][core, iloc]  # [K]
            M = c_q[lab]  # [K, 256]
            M2 = M.reshape(K, 2, P).transpose(2, 1, 0)  # [128, 2, K]
            parts.append(M2.reshape(P, 2 * K))
        cents_maps.append(np.ascontiguousarray(np.concatenate(parts, axis=1)))

    # ---- ones pack: per size [128, K] block-indicator ----
    ow = []
    for s in SIZES:
        if t_by_size[s] == 0:
            continue
        K = P // s
        blk = np.zeros((P, K), dtype=FP8)
        for jj in range(K):
            blk[jj * s : (jj + 1) * s, jj] = 1.0
        ow.append(blk)
    onesp = np.ascontiguousarray(np.concatenate(ow, axis=1))

    ident = np.eye(P, dtype=ml_dtypes.bfloat16)

    maps = []
    fview = fpad.reshape(N_CORES, P, T * D)
    for core in range(N_CORES):
        maps.append(
            {
                "features": np.ascontiguousarray(fview[core]),
                "cents": cents_maps[core],
                "onesp": onesp,
                "ident": ident,
            }
        )

    # ---- host-exact center term over all device rows (incl pads) ----
    cn2 = (c_qf * c_qf).sum(axis=1)  # fp64 ||c_q||^2 per class
    c2_total = 0.0
    for s in SIZES:
        if len(bl[s]):
            c2_total += s * float(cn2[bl[s]].sum())

    return maps, cfg, c2_total


def run(features, centers, labels, trace=False):
    maps, cfg, c2_total = _prepare(features, centers, labels)
    nc, NBANK = _build(cfg)
    res = run_bass_kernel_spmd(nc, maps, core_ids=list(range(N_CORES)), trace=trace)
    gram = 0.0
    cross = 0.0
    for r in res.results:
        o = np.asarray(r["out"]).astype(np.float64)
        gram += float(o[:, 0].sum())
        cross += float(o[:, 1 : 2 * NBANK : 2].sum())
    total = gram - 2.0 * cross + c2_total
    return np.float32(total / B), res


def kernel(features, centers, labels):
    last_err = None
    for _ in range(3):
        try:
            loss, _ = run(features, centers, labels)
            return loss
        except Exception as e:  # noqa: BLE001
            last_err = e
    raise last_err


# revision 4
# speedup vs baseline: 1.0945x; 1.0269x over previous
"""AdaptiveCenterLoss on 8 TRN2 NeuronCores — fp8 flipped-layout version.

loss = mean_i ||features[i] - centers[labels[i]]||^2, B=131072 D=256 C=1000.

Strategy (memory-bound; tolerance 2e-2 >> fp8 e4m3 error ~1e-3):
  loss*B = sum(f^2) - 2*sum_i f_i.c_{l_i} + sum_i ||c_{l_i}||^2
  - features ship as fp8 e4m3 (4.2MB/core instead of 8.4MB bf16).
  - layout is FLIPPED vs the bf16 kernel: rows sit on PARTITIONS.  Each
    tile is [128 rows x 256 d]; host sorts rows by label and packs them
    so each tile is made of single-label blocks of s rows (s in
    {16,8,4,2,1} by binary decomposition of class counts; pad rows equal
    the class-0 center so they cancel exactly).
  - TensorE does ALL the heavy math in one pass: per 128-col chunk X,
    matmul(X, X) accumulates a Gram whose diag sums to sum(f^2), and a
    tiny matmul(X, ones_blocks[128,k]) emits per-block column sums
    (k = 128/s).  Both land in the SAME PSUM bank ("combo" layout:
    cols 0:128 gram, 128:512 colsums) so back-to-back matmuls never
    switch PSUM banks — bank switches were measured to cost ~25ns each
    on the MM pipe.  Tiles are grouped into 7 stretches, one bank each.
  - When a stretch completes, DVE dots its colsums against the per-block
    centers (fp8, host-materialized in PSUM column order) and extracts
    its partial Gram diagonal via an identity mask — work spreads
    through the run instead of piling at the drain.
  - host computes sum ||c||^2 exactly in fp64 and combines.
"""

import numpy as np
import ml_dtypes

import concourse.bacc as bacc
import concourse.bass as bass
import concourse.mybir as mybir
import concourse.tile as tile
from concourse.bass_utils import run_bass_kernel_spmd

B, D, C = 131072, 256, 1000
N_CORES = 8
P = 128
SIZES = (1, 2, 4, 8, 16)  # region order on device (expensive colsums first)
FP8 = ml_dtypes.float8_e4m3
CSCAP = 512  # colsum columns per PSUM bank

_nc_cache = {}


def _slice_plan(T):
    """DMA slices: small first (compute starts early), big middle,
    small last (short drain tail)."""
    sizes = []
    rem = T
    for want in (3, 5, 8, 12, 16, 20):
        if rem <= 0:
            break
        take = min(want, rem)
        sizes.append(take)
        rem -= take
    mid = []
    while rem > 18:
        take = min(24, rem - 12)
        mid.append(take)
        rem -= take
    tail = []
    while rem > 0:
        take = min(8, rem)
        tail.append(take)
        rem -= take
    return sizes + mid + sorted(tail, reverse=True)


def _banks(tile_size):
    """Greedy stretch assignment: per tile, its 2k colsum cols must fit
    in the current bank's colsum capacity; else open a new bank.
    Returns per-tile (bank, col offset within bank's colsum area)."""
    assign = []
    bank, off = 0, 0
    for s in tile_size:
        w = 2 * (P // s)
        if off + w > CSCAP:
            bank += 1
            off = 0
        assign.append((bank, off))
        off += w
    return assign, bank + 1


def _build(cfg):
    if cfg in _nc_cache:
        return _nc_cache[cfg]
    t_by_size = dict(zip(SIZES, cfg))
    T = sum(t_by_size.values())
    tile_size = []
    for s in SIZES:
        tile_size += [s] * t_by_size[s]
    assign, NBANK = _banks(tile_size)
    assert NBANK <= 7, f"{NBANK} colsum banks + gram exceed PSUM"
    OW = sum(P // s for s in SIZES if t_by_size[s])  # ones pack width
    slices = _slice_plan(T)
    # colsum column counts per bank (for cents slicing)
    bank_w = [0] * NBANK
    for (b, off), s in zip(assign, tile_size):
        bank_w[b] = max(bank_w[b], off + 2 * (P // s))
    bank_c0 = np.concatenate(([0], np.cumsum(bank_w)[:-1])).tolist()
    CC = int(sum(bank_w))

    nc = bacc.Bacc()
    feats = nc.declare_dram_parameter(
        "features", [P, T * D], mybir.dt.float8e4, isOutput=False
    )
    cents = nc.declare_dram_parameter(
        "cents", [P, CC], mybir.dt.float8e4, isOutput=False
    )
    onesp = nc.declare_dram_parameter(
        "onesp", [P, OW], mybir.dt.float8e4, isOutput=False
    )
    ident = nc.declare_dram_parameter("ident", [P, P], mybir.dt.bfloat16, isOutput=False)
    out = nc.declare_dram_parameter("out", [P, 16], mybir.dt.float32, isOutput=True)

    # ones views offsets per size
    ones_off = {}
    o = 0
    for s in SIZES:
        if t_by_size[s]:
            ones_off[s] = o
            o += P // s

    with tile.TileContext(nc) as tc:
        with (
            tc.tile_pool(name="c", bufs=1) as c_pool,
            tc.tile_pool(name="x", bufs=len(slices)) as x_pool,
            tc.tile_pool(name="scr", bufs=2) as scr_pool,
            tc.tile_pool(name="ps", bufs=1, space=bass.MemorySpace.PSUM) as ps_pool,
        ):
            ones_t = c_pool.tile([P, OW], mybir.dt.float8e4)
            nc.scalar.dma_start(out=ones_t[:], in_=onesp[:])
            idt = c_pool.tile([P, P], mybir.dt.bfloat16)
            nc.scalar.dma_start(out=idt[:], in_=ident[:])
            cents_t = c_pool.tile([P, CC], mybir.dt.float8e4)
            nc.scalar.dma_start(out=cents_t[:], in_=cents[:])
            acc = c_pool.tile([P, 16], mybir.dt.float32)

            ps_gram = ps_pool.tile([P, P], mybir.dt.float32, tag="g")
            combo = [
                ps_pool.tile([P, 512], mybir.dt.float32, tag=f"cb{i}", name=f"cb{i}")
                for i in range(NBANK)
            ]

            # feature slices
            xs = []
            t0 = 0
            for i, nt in enumerate(slices):
                xt = x_pool.tile([P, nt * D], mybir.dt.float8e4, tag="x", name=f"x{i}")
                nc.sync.dma_start(out=xt[:], in_=feats[:, t0 * D : (t0 + nt) * D])
                xs.append((t0, nt, xt))
                t0 += nt

            # first/last tile of each bank stretch
            first_of = {}
            last_of = {}
            for tl, (b, _) in enumerate(assign):
                if b not in first_of:
                    first_of[b] = tl
                last_of[b] = tl

            def finish_bank(b):
                w = bank_w[b]
                scr = scr_pool.tile([P, 512], mybir.dt.float32, tag="scr")
                nc.vector.scalar_tensor_tensor(
                    out=scr[:, :w],
                    in0=combo[b][:, :w],
                    scalar=0.0,
                    in1=cents_t[:, bank_c0[b] : bank_c0[b] + w],
                    op0=mybir.AluOpType.bypass,
                    op1=mybir.AluOpType.mult,
                    accum_out=acc[:, 2 * b + 1 : 2 * b + 2],
                )

            # slices whose tiles compute sum(f^2) on the (otherwise idle)
            # ACT engine via Square+accum instead of the TE gram matmul
            act_slices = {
                si
                for si, (s0, nt) in enumerate(
                    (t0_, nt) for t0_, nt, _ in xs
                )
                if 5 <= si <= 6
            }
            n_te_gram = 2 * sum(
                nt for si, (t0_, nt, _) in enumerate(xs) if si not in act_slices
            )
            acol = 2  # even acc columns 2,4,... for ACT partial sums
            ci = 0
            for si, (t0_, nt, xt) in enumerate(xs):
                is_act = si in act_slices
                for tl in range(t0_, t0_ + nt):
                    s = tile_size[tl]
                    k = P // s
                    base = (tl - t0_) * D
                    b, off = assign[tl]
                    for h in range(2):
                        chunk = xt[:, base + h * P : base + (h + 1) * P]
                        if not is_act:
                            nc.tensor.matmul(
                                ps_gram[:],
                                chunk,
                                chunk,
                                start=(ci == 0),
                                stop=(ci == n_te_gram - 1),
                            )
                            ci += 1
                        nc.tensor.matmul(
                            combo[b][:, off + h * k : off + (h + 1) * k],
                            chunk,
                            ones_t[:, ones_off[s] : ones_off[s] + k],
                            start=True,
                            stop=True,
                        )
                    if tl == last_of[b]:
                        finish_bank(b)
                if is_act:
                    done = 0
                    while done < nt:
                        g = min(8, nt - done)
                        scra = scr_pool.tile(
                            [P, 8 * D], mybir.dt.float32, tag="scra"
                        )
                        nc.scalar.activation(
                            out=scra[:, : g * D],
                            in_=xt[:, done * D : (done + g) * D],
                            func=mybir.ActivationFunctionType.Square,
                            accum_out=acc[:, acol : acol + 1],
                        )
                        acol += 2
                        done += g
            # gram diagonal
            scr2 = scr_pool.tile([P, P], mybir.dt.float32, tag="scr2")
            nc.vector.scalar_tensor_tensor(
                out=scr2[:],
                in0=ps_gram[:],
                scalar=0.0,
                in1=idt[:],
                op0=mybir.AluOpType.bypass,
                op1=mybir.AluOpType.mult,
                accum_out=acc[:, 0:1],
            )
            nc.sync.dma_start(out=out[:], in_=acc[:])
    nc.finalize()
    _nc_cache[cfg] = (nc, NBANK)
    return nc, NBANK


def _prepare(features, centers, labels):
    features = np.asarray(features)
    centers_f = np.ascontiguousarray(np.asarray(centers), dtype=np.float32)
    c_q = centers_f.astype(FP8)  # quantized centers (used consistently)
    c_qf = c_q.astype(np.float64)
    labels = np.asarray(labels).astype(np.int64)

    counts = np.bincount(labels, minlength=C).astype(np.int64)
    nb = {16: counts // 16}
    rem = counts % 16
    nb[8] = (rem // 8) % 2
    nb[4] = (rem // 4) % 2
    nb[2] = (rem // 2) % 2
    nb[1] = rem % 2

    # per-core tile counts, padded block lists (pad class = 0)
    t_by_size = {}
    bl = {}
    for s in SIZES:
        K = P // s
        N_s = int(nb[s].sum())
        t_s = -(-N_s // (N_CORES * K)) if N_s else 0
        t_by_size[s] = t_s
        cap = N_CORES * t_s * K
        b = np.repeat(np.arange(C, dtype=np.int64), nb[s])
        if cap > len(b):
            b = np.concatenate([b, np.zeros(cap - len(b), dtype=np.int64)])
        bl[s] = b
    T = sum(t_by_size.values())
    cfg = tuple(t_by_size[s] for s in SIZES)
    tile_size = []
    for s in SIZES:
        tile_size += [s] * t_by_size[s]
    assign, NBANK = _banks(tile_size)

    # region tile offsets (per core), region order = SIZES
    region_off = {}
    o = 0
    for s in SIZES:
        region_off[s] = o
        o += t_by_size[s]

    # ---- row destinations ----
    order_idx = np.argsort(labels, kind="stable")
    labels_sorted = labels[order_idx]
    class_row_start = np.concatenate(([0], np.cumsum(counts)[:-1]))
    rank = np.arange(B, dtype=np.int64) - class_row_start[labels_sorted]

    dst = np.empty(B, dtype=np.int64)
    lo = np.zeros(C, dtype=np.int64)
    for s in SIZES:
        ns = nb[s]
        hi = lo + s * ns
        m = (rank >= lo[labels_sorted]) & (rank < hi[labels_sorted])
        if m.any():
            K = P // s
            j = labels_sorted[m]
            r = rank[m] - lo[j]
            start_s = np.concatenate(([0], np.cumsum(ns)[:-1]))
            bidx = start_s[j] + r // s  # global block index in bl[s]
            per_core = t_by_size[s] * K
            core = bidx // per_core
            loc = bidx % per_core
            tile_loc = loc // K
            bslot = loc % K
            part = bslot * s + r % s
            tile_glob = region_off[s] + tile_loc
            dst[m] = (core * P + part) * T + tile_glob
        lo = hi

    f_q = features.astype(np.float32).astype(FP8)
    fpad = np.empty((N_CORES * P * T, D), dtype=FP8)
    fpad[:] = c_q[0]  # pad rows cancel exactly
    fpad[dst] = f_q[order_idx]

    # ---- centsT (per-core, PSUM combo-bank column order) ----
    # within a bank, columns follow tile order (h0 k cols then h1 k cols),
    # padded to bank_w at the end of the bank; pad cols get 0 centers
    # (their PSUM cells are never written... but also never dotted since
    # bank_w only covers written columns).
    lab_by_size = {}
    for s in SIZES:
        if t_by_size[s]:
            lab_by_size[s] = bl[s].reshape(N_CORES, t_by_size[s], P // s)
    tinfo = []
    for s in SIZES:
        for i in range(t_by_size[s]):
            tinfo.append((s, i))

    cents_maps = []
    for core in range(N_CORES):
        parts = []
        for tl in range(T):
            s, iloc = tinfo[tl]
            K = P // s
            lab = lab_by_size[s][core, iloc]  # [K]
            M = c_q[lab]  # [K, 256]
            M2 = M.reshape(K, 2, P).transpose(2, 1, 0)  # [128, 2, K]
            parts.append(M2.reshape(P, 2 * K))
        cents_maps.append(np.ascontiguousarray(np.concatenate(parts, axis=1)))

    # ---- ones pack: per size [128, K] block-indicator ----
    ow = []
    for s in SIZES:
        if t_by_size[s] == 0:
            continue
        K = P // s
        blk = np.zeros((P, K), dtype=FP8)
        for jj in range(K):
            blk[jj * s : (jj + 1) * s, jj] = 1.0
        ow.append(blk)
    onesp = np.ascontiguousarray(np.concatenate(ow, axis=1))

    ident = np.eye(P, dtype=ml_dtypes.bfloat16)

    maps = []
    fview = fpad.reshape(N_CORES, P, T * D)
    for core in range(N_CORES):
        maps.append(
            {
                "features": np.ascontiguousarray(fview[core]),
                "cents": cents_maps[core],
                "onesp": onesp,
                "ident": ident,
            }
        )

    # ---- host-exact center term over all device rows (incl pads) ----
    cn2 = (c_qf * c_qf).sum(axis=1)  # fp64 ||c_q||^2 per class
    c2_total = 0.0
    for s in SIZES:
        if len(bl[s]):
            c2_total += s * float(cn2[bl[s]].sum())

    return maps, cfg, c2_total


def run(features, centers, labels, trace=False):
    maps, cfg, c2_total = _prepare(features, centers, labels)
    nc, NBANK = _build(cfg)
    res = run_bass_kernel_spmd(nc, maps, core_ids=list(range(N_CORES)), trace=trace)
    gram = 0.0
    cross = 0.0
    for r in res.results:
        o = np.asarray(r["out"]).astype(np.float64)
        gram += float(o[:, 0].sum())
        gram += float(o[:, 2:16:2].sum())
        cross += float(o[:, 1 : 2 * NBANK : 2].sum())
    total = gram - 2.0 * cross + c2_total
    return np.float32(total / B), res


def kernel(features, centers, labels):
    last_err = None
    for _ in range(3):
        try:
            loss, _ = run(features, centers, labels)
            return loss
        except Exception as e:  # noqa: BLE001
            last_err = e
    raise last_err


# revision 5
# speedup vs baseline: 1.1379x; 1.0396x over previous
"""AdaptiveCenterLoss on 8 TRN2 NeuronCores — fp8 flipped-layout version.

loss = mean_i ||features[i] - centers[labels[i]]||^2, B=131072 D=256 C=1000.

Strategy (memory-bound; tolerance 2e-2 >> fp8 e4m3 error ~1e-3):
  loss*B = sum(f^2) - 2*sum_i f_i.c_{l_i} + sum_i ||c_{l_i}||^2
  - features ship as fp8 e4m3 (4.2MB/core instead of 8.4MB bf16).
  - layout is FLIPPED vs the bf16 kernel: rows sit on PARTITIONS.  Each
    tile is [128 rows x 256 d]; host sorts rows by label and packs them
    so each tile is made of single-label blocks of s rows (s in
    {16,8,4,2,1} by binary decomposition of class counts; pad rows equal
    the class-0 center so they cancel exactly).
  - TensorE does ALL the heavy math in one pass: per 128-col chunk X,
    matmul(X, X) accumulates a Gram whose diag sums to sum(f^2), and a
    tiny matmul(X, ones_blocks[128,k]) emits per-block column sums
    (k = 128/s).  Both land in the SAME PSUM bank ("combo" layout:
    cols 0:128 gram, 128:512 colsums) so back-to-back matmuls never
    switch PSUM banks — bank switches were measured to cost ~25ns each
    on the MM pipe.  Tiles are grouped into 7 stretches, one bank each.
  - When a stretch completes, DVE dots its colsums against the per-block
    centers (fp8, host-materialized in PSUM column order) and extracts
    its partial Gram diagonal via an identity mask — work spreads
    through the run instead of piling at the drain.
  - host computes sum ||c||^2 exactly in fp64 and combines.
"""

import numpy as np
import ml_dtypes

import concourse.bacc as bacc
import concourse.bass as bass
import concourse.mybir as mybir
import concourse.tile as tile
from concourse.bass_utils import run_bass_kernel_spmd

B, D, C = 131072, 256, 1000
N_CORES = 8
P = 128
SIZES = (1, 2, 4, 8, 16)  # region order on device (expensive colsums first)
FP8 = ml_dtypes.float8_e4m3
CSCAP = 512  # colsum columns per PSUM bank

_nc_cache = {}


def _slice_plan(T):
    """DMA slices: small first (compute starts early), big middle,
    small last (short drain tail)."""
    sizes = []
    rem = T
    for want in (3, 5, 8, 12, 16, 20):
        if rem <= 0:
            break
        take = min(want, rem)
        sizes.append(take)
        rem -= take
    mid = []
    while rem > 18:
        take = min(24, rem - 12)
        mid.append(take)
        rem -= take
    tail = []
    while rem > 0:
        take = min(8, rem)
        tail.append(take)
        rem -= take
    return sizes + mid + sorted(tail, reverse=True)


def _banks(tile_size):
    """Greedy stretch assignment: per tile, its 2k colsum cols must fit
    in the current bank's colsum capacity; else open a new bank.
    Returns per-tile (bank, col offset within bank's colsum area)."""
    assign = []
    bank, off = 0, 0
    for s in tile_size:
        w = 2 * (P // s)
        if off + w > CSCAP:
            bank += 1
            off = 0
        assign.append((bank, off))
        off += w
    return assign, bank + 1


def _build(cfg):
    if cfg in _nc_cache:
        return _nc_cache[cfg]
    t_by_size = dict(zip(SIZES, cfg))
    T = sum(t_by_size.values())
    tile_size = []
    for s in SIZES:
        tile_size += [s] * t_by_size[s]
    assign, NBANK = _banks(tile_size)
    assert NBANK <= 7, f"{NBANK} colsum banks + gram exceed PSUM"
    OW = sum(P // s for s in SIZES if t_by_size[s])  # ones pack width
    slices = _slice_plan(T)
    # colsum column counts per bank (for cents slicing)
    bank_w = [0] * NBANK
    for (b, off), s in zip(assign, tile_size):
        bank_w[b] = max(bank_w[b], off + 2 * (P // s))
    bank_c0 = np.concatenate(([0], np.cumsum(bank_w)[:-1])).tolist()
    CC = int(sum(bank_w))

    nc = bacc.Bacc()
    feats = nc.declare_dram_parameter(
        "features", [P, T * D], mybir.dt.float8e4, isOutput=False
    )
    cents = nc.declare_dram_parameter(
        "cents", [P, CC], mybir.dt.float8e4, isOutput=False
    )
    onesp = nc.declare_dram_parameter(
        "onesp", [P, OW], mybir.dt.float8e4, isOutput=False
    )
    ident = nc.declare_dram_parameter("ident", [P, P], mybir.dt.bfloat16, isOutput=False)
    out = nc.declare_dram_parameter("out", [P, 16], mybir.dt.float32, isOutput=True)

    # ones views offsets per size
    ones_off = {}
    o = 0
    for s in SIZES:
        if t_by_size[s]:
            ones_off[s] = o
            o += P // s

    with tile.TileContext(nc) as tc:
        with (
            tc.tile_pool(name="c", bufs=1) as c_pool,
            tc.tile_pool(name="x", bufs=len(slices)) as x_pool,
            tc.tile_pool(name="scr", bufs=2) as scr_pool,
            tc.tile_pool(name="ps", bufs=1, space=bass.MemorySpace.PSUM) as ps_pool,
        ):
            ones_t = c_pool.tile([P, OW], mybir.dt.float8e4)
            nc.scalar.dma_start(out=ones_t[:], in_=onesp[:])
            idt = c_pool.tile([P, P], mybir.dt.bfloat16)
            nc.scalar.dma_start(out=idt[:], in_=ident[:])
            cents_t = c_pool.tile([P, CC], mybir.dt.float8e4)
            nc.scalar.dma_start(out=cents_t[:], in_=cents[:])
            acc = c_pool.tile([P, 16], mybir.dt.float32)

            ps_gram = ps_pool.tile([P, P], mybir.dt.float32, tag="g")
            combo = [
                ps_pool.tile([P, 512], mybir.dt.float32, tag=f"cb{i}", name=f"cb{i}")
                for i in range(NBANK)
            ]

            # feature slices
            xs = []
            t0 = 0
            for i, nt in enumerate(slices):
                xt = x_pool.tile([P, nt * D], mybir.dt.float8e4, tag="x", name=f"x{i}")
                nc.sync.dma_start(out=xt[:], in_=feats[:, t0 * D : (t0 + nt) * D])
                xs.append((t0, nt, xt))
                t0 += nt

            # first/last tile of each bank stretch
            first_of = {}
            last_of = {}
            for tl, (b, _) in enumerate(assign):
                if b not in first_of:
                    first_of[b] = tl
                last_of[b] = tl

            def finish_bank(b):
                w = bank_w[b]
                scr = scr_pool.tile([P, 512], mybir.dt.float32, tag="scr")
                nc.vector.scalar_tensor_tensor(
                    out=scr[:, :w],
                    in0=combo[b][:, :w],
                    scalar=0.0,
                    in1=cents_t[:, bank_c0[b] : bank_c0[b] + w],
                    op0=mybir.AluOpType.bypass,
                    op1=mybir.AluOpType.mult,
                    accum_out=acc[:, 2 * b + 1 : 2 * b + 2],
                )

            # slices whose tiles compute sum(f^2) on the (otherwise idle)
            # ACT engine via Square+accum instead of the TE gram matmul
            act_slices = {
                si
                for si, (s0, nt) in enumerate(
                    (t0_, nt) for t0_, nt, _ in xs
                )
                if 5 <= si <= 6
            }
            DVE_SL, DVE_N = 7, 12  # slice 7's first 12 tiles: gram on DVE
            n_te_gram = 2 * (
                sum(nt for si, (t0_, nt, _) in enumerate(xs) if si not in act_slices)
                - DVE_N
            )
            acol = 2  # even acc columns 2,4,... for ACT partial sums
            ci = 0
            for si, (t0_, nt, xt) in enumerate(xs):
                is_act = si in act_slices
                for tl in range(t0_, t0_ + nt):
                    s = tile_size[tl]
                    k = P // s
                    base = (tl - t0_) * D
                    b, off = assign[tl]
                    is_dve = si == DVE_SL and (tl - t0_) < DVE_N
                    for h in range(2):
                        chunk = xt[:, base + h * P : base + (h + 1) * P]
                        if not (is_act or is_dve):
                            nc.tensor.matmul(
                                ps_gram[:],
                                chunk,
                                chunk,
                                start=(ci == 0),
                                stop=(ci == n_te_gram - 1),
                            )
                            ci += 1
                        nc.tensor.matmul(
                            combo[b][:, off + h * k : off + (h + 1) * k],
                            chunk,
                            ones_t[:, ones_off[s] : ones_off[s] + k],
                            start=True,
                            stop=True,
                        )
                    if tl == last_of[b]:
                        finish_bank(b)
                if si == DVE_SL:
                    xv = xt[:, 0 : DVE_N * D]
                    scrv = scr_pool.tile([P, DVE_N * D], mybir.dt.float8e4, tag="scrv")
                    nc.vector.scalar_tensor_tensor(
                        out=scrv[:],
                        in0=xv,
                        scalar=0.0,
                        in1=xv,
                        op0=mybir.AluOpType.bypass,
                        op1=mybir.AluOpType.mult,
                        accum_out=acc[:, 14:15],
                    )
                if is_act:
                    done = 0
                    while done < nt:
                        g = min(8, nt - done)
                        scra = scr_pool.tile(
                            [P, 8 * D], mybir.dt.float32, tag="scra"
                        )
                        nc.scalar.activation(
                            out=scra[:, : g * D],
                            in_=xt[:, done * D : (done + g) * D],
                            func=mybir.ActivationFunctionType.Square,
                            accum_out=acc[:, acol : acol + 1],
                        )
                        acol += 2
                        done += g
            # gram diagonal
            scr2 = scr_pool.tile([P, P], mybir.dt.float32, tag="scr2")
            nc.vector.scalar_tensor_tensor(
                out=scr2[:],
                in0=ps_gram[:],
                scalar=0.0,
                in1=idt[:],
                op0=mybir.AluOpType.bypass,
                op1=mybir.AluOpType.mult,
                accum_out=acc[:, 0:1],
            )
            nc.sync.dma_start(out=out[:], in_=acc[:])
    nc.finalize()
    _nc_cache[cfg] = (nc, NBANK)
    return nc, NBANK


def _prepare(features, centers, labels):
    features = np.asarray(features)
    centers_f = np.ascontiguousarray(np.asarray(centers), dtype=np.float32)
    c_q = centers_f.astype(FP8)  # quantized centers (used consistently)
    c_qf = c_q.astype(np.float64)
    labels = np.asarray(labels).astype(np.int64)

    counts = np.bincount(labels, minlength=C).astype(np.int64)
    nb = {16: counts // 16}
    rem = counts % 16
    nb[8] = (rem // 8) % 2
    nb[4] = (rem // 4) % 2
    nb[2] = (rem // 2) % 2
    nb[1] = rem % 2

    # per-core tile counts, padded block lists (pad class = 0)
    t_by_size = {}
    bl = {}
    for s in SIZES:
        K = P // s
        N_s = int(nb[s].sum())
        t_s = -(-N_s // (N_CORES * K)) if N_s else 0
        t_by_size[s] = t_s
        cap = N_CORES * t_s * K
        b = np.repeat(np.arange(C, dtype=np.int64), nb[s])
        if cap > len(b):
            b = np.concatenate([b, np.zeros(cap - len(b), dtype=np.int64)])
        bl[s] = b
    T = sum(t_by_size.values())
    cfg = tuple(t_by_size[s] for s in SIZES)
    tile_size = []
    for s in SIZES:
        tile_size += [s] * t_by_size[s]
    assign, NBANK = _banks(tile_size)

    # region tile offsets (per core), region order = SIZES
    region_off = {}
    o = 0
    for s in SIZES:
        region_off[s] = o
        o += t_by_size[s]

    # ---- row destinations ----
    order_idx = np.argsort(labels, kind="stable")
    labels_sorted = labels[order_idx]
    class_row_start = np.concatenate(([0], np.cumsum(counts)[:-1]))
    rank = np.arange(B, dtype=np.int64) - class_row_start[labels_sorted]

    dst = np.empty(B, dtype=np.int64)
    lo = np.zeros(C, dtype=np.int64)
    for s in SIZES:
        ns = nb[s]
        hi = lo + s * ns
        m = (rank >= lo[labels_sorted]) & (rank < hi[labels_sorted])
        if m.any():
            K = P // s
            j = labels_sorted[m]
            r = rank[m] - lo[j]
            start_s = np.concatenate(([0], np.cumsum(ns)[:-1]))
            bidx = start_s[j] + r // s  # global block index in bl[s]
            per_core = t_by_size[s] * K
            core = bidx // per_core
            loc = bidx % per_core
            tile_loc = loc // K
            bslot = loc % K
            part = bslot * s + r % s
            tile_glob = region_off[s] + tile_loc
            dst[m] = (core * P + part) * T + tile_glob
        lo = hi

    f_q = features.astype(np.float32).astype(FP8)
    fpad = np.empty((N_CORES * P * T, D), dtype=FP8)
    fpad[:] = c_q[0]  # pad rows cancel exactly
    fpad[dst] = f_q[order_idx]

    # ---- centsT (per-core, PSUM combo-bank column order) ----
    # within a bank, columns follow tile order (h0 k cols then h1 k cols),
    # padded to bank_w at the end of the bank; pad cols get 0 centers
    # (their PSUM cells are never written... but also never dotted since
    # bank_w only covers written columns).
    lab_by_size = {}
    for s in SIZES:
        if t_by_size[s]:
            lab_by_size[s] = bl[s].reshape(N_CORES, t_by_size[s], P // s)
    tinfo = []
    for s in SIZES:
        for i in range(t_by_size[s]):
            tinfo.append((s, i))

    cents_maps = []
    for core in range(N_CORES):
        parts = []
        for tl in range(T):
            s, iloc = tinfo[tl]
            K = P // s
            lab = lab_by_size[s][core, iloc]  # [K]
            M = c_q[lab]  # [K, 256]
            M2 = M.reshape(K, 2, P).transpose(2, 1, 0)  # [128, 2, K]
            parts.append(M2.reshape(P, 2 * K))
        cents_maps.append(np.ascontiguousarray(np.concatenate(parts, axis=1)))

    # ---- ones pack: per size [128, K] block-indicator ----
    ow = []
    for s in SIZES:
        if t_by_size[s] == 0:
            continue
        K = P // s
        blk = np.zeros((P, K), dtype=FP8)
        for jj in range(K):
            blk[jj * s : (jj + 1) * s, jj] = 1.0
        ow.append(blk)
    onesp = np.ascontiguousarray(np.concatenate(ow, axis=1))

    ident = np.eye(P, dtype=ml_dtypes.bfloat16)

    maps = []
    fview = fpad.reshape(N_CORES, P, T * D)
    for core in range(N_CORES):
        maps.append(
            {
                "features": np.ascontiguousarray(fview[core]),
                "cents": cents_maps[core],
                "onesp": onesp,
                "ident": ident,
            }
        )

    # ---- host-exact center term over all device rows (incl pads) ----
    cn2 = (c_qf * c_qf).sum(axis=1)  # fp64 ||c_q||^2 per class
    c2_total = 0.0
    for s in SIZES:
        if len(bl[s]):
            c2_total += s * float(cn2[bl[s]].sum())

    return maps, cfg, c2_total


def run(features, centers, labels, trace=False):
    maps, cfg, c2_total = _prepare(features, centers, labels)
    nc, NBANK = _build(cfg)
    res = run_bass_kernel_spmd(nc, maps, core_ids=list(range(N_CORES)), trace=trace)
    gram = 0.0
    cross = 0.0
    for r in res.results:
        o = np.asarray(r["out"]).astype(np.float64)
        gram += float(o[:, 0].sum())
        gram += float(o[:, 2:16:2].sum())
        cross += float(o[:, 1 : 2 * NBANK : 2].sum())
    total = gram - 2.0 * cross + c2_total
    return np.float32(total / B), res


def kernel(features, centers, labels):
    last_err = None
    for _ in range(3):
        try:
            loss, _ = run(features, centers, labels)
            return loss
        except Exception as e:  # noqa: BLE001
            last_err = e
    raise last_err
